# revision 13
# baseline (speedup 1.0000x reference)
"""Trainium2 Bass kernel for COMETGate MoE routing.

Computes, for each row b:
    s      = smoothstep(x @ Wz + bz)                       [B, 15, 8]
    prob   = binary-tree path products of s                [B, 16, 8]
    a      = x @ Ww + bw                                   [B, 16, 8]
    score  = a + (prob > 0 ? log(prob + 1e-8) : -inf)      [B, 16*8]
    e      = exp(score - max(score))        (softmax numerator)
    u[l]   = sum_j e[j] * P[j, l]           (permutation mix)
    g      = u / sum(u)                                    [B, 16]
    y[b,d] = sum_n f[b, d, n] * g[b, n]                    [B, 1024]

Sharding: data-parallel over B across 8 NeuronCores (1024 rows each).
Host-side prep: f transposed to [B, N, D] (unit-stride expert slices),
x transposed to [D, B] (matmul stationary wants contraction on
partitions), Wz/Ww fused into one [1024, 248] rhs matrix.
"""

import sys

for _p in ("/opt/trn_rl_repo", "/root/.axon_site/_ro/trn_rl_repo"):
    if _p not in sys.path:
        sys.path.insert(0, _p)

import numpy as np

import concourse.bass as bass
import concourse.tile as tile
from concourse import bacc, mybir

F32 = mybir.dt.float32
ALU = mybir.AluOpType
ACTF = mybir.ActivationFunctionType

B, D_IN, D_OUT = 8192, 1024, 1024
N_EXP, K_TREE = 16, 8
N_CORES = 8
BS = B // N_CORES          # 1024 rows per core
NB = BS // 128             # 8 b-tiles of 128 rows
NZ = (N_EXP - 1) * K_TREE  # 120 selector columns
NW = N_EXP * K_TREE        # 128 leaf columns
NM = NZ + NW               # 248 fused matmul outputs
DH = 512                   # d-chunk for the weighting stage

_CACHED_NC = None
LAST_RESULTS = None  # BassKernelResults of the most recent run (for test.py)


def build_nc():
    nc = bacc.Bacc("TRN2", target_bir_lowering=False, debug=False)

    ft = nc.dram_tensor("ft", [BS, N_EXP, D_OUT], F32, kind="ExternalInput").ap()
    xw = nc.dram_tensor("xw", [D_IN, BS + NM], F32, kind="ExternalInput").ap()
    biasv = nc.dram_tensor("biasv", [NM], F32, kind="ExternalInput").ap()
    pmatt = nc.dram_tensor("pmatt", [N_EXP, NW], F32, kind="ExternalInput").ap()
    y = nc.dram_tensor("y", [BS, D_OUT], F32, kind="ExternalOutput").ap()

    with tile.TileContext(nc) as tc:
        with (
            tc.tile_pool(name="singles", bufs=1) as singles,
            tc.tile_pool(name="work", bufs=2) as work,
            tc.tile_pool(name="fpool", bufs=3) as fpool,
            tc.tile_pool(name="ypool", bufs=2) as ypool,
            tc.tile_pool(name="psum", bufs=2, space="PSUM") as psum,
        ):
            # ---- resident constants ----
            xw_sb = singles.tile([128, D_IN // 128, BS + NM], F32)  # [p, kc, b|m]
            nc.sync.dma_start(
                out=xw_sb, in_=xw.rearrange("(c p) m -> p c m", p=128)
            )
            bias_sb = singles.tile([128, NM], F32)
            bias_ap = biasv[:]
            bias_bc = bass.AP(
                tensor=bias_ap.tensor,
                offset=bias_ap.offset,
                ap=[[0, 128]] + list(bias_ap.ap),
            )
            nc.gpsimd.dma_start(out=bias_sb, in_=bias_bc)
            pmat_sb = singles.tile([128, N_EXP, NW], F32)
            pmat_ap = pmatt[:, :]
            pmat_bc = bass.AP(
                tensor=pmat_ap.tensor,
                offset=pmat_ap.offset,
                ap=[[0, 128]] + list(pmat_ap.ap),
            )
            nc.gpsimd.dma_start(out=pmat_sb, in_=pmat_bc)
            eps_sb = singles.tile([128, 1], F32)
            nc.vector.memset(eps_sb, 1e-8)
            # Wait-absorbers: HW instructions carry a single sync-wait slot,
            # so let DVE observe each input DMA once, up front; later DVE
            # consumers then need no additional cross-engine waits.
            absorb = singles.tile([128, 1], F32)
            nc.vector.tensor_copy(absorb, bias_sb[:, 0:1])
            nc.vector.tensor_copy(absorb, pmat_sb[:, 0, 0:1])
            nc.vector.tensor_copy(absorb, xw_sb[:, 0, 0:1])

            for bt in range(NB):
                bsl = slice(bt * 128, (bt + 1) * 128)

                # ---- routing matmul: scores[b, m] = sum_d x[b,d] W[d,m] ----
                sc_ps = psum.tile([128, NM], F32)
                for kc in range(D_IN // 128):
                    nc.tensor.matmul(
                        sc_ps,
                        xw_sb[:, kc, bsl],
                        xw_sb[:, kc, BS:BS + NM],
                        start=(kc == 0),
                        stop=(kc == D_IN // 128 - 1),
                    )
                zall = work.tile([128, NM], F32)
                nc.vector.tensor_add(zall, sc_ps, bias_sb)

                # ---- smoothstep: s = poly(clamp(z, -.5, .5)) ----
                z = zall[:, 0:NZ]
                zc = work.tile([128, NZ], F32)
                nc.vector.tensor_scalar(
                    out=zc, in0=z, scalar1=-0.5, scalar2=0.5,
                    op0=ALU.max, op1=ALU.min,
                )
                z2 = work.tile([128, NZ], F32)
                nc.vector.tensor_mul(z2, zc, zc)
                t2 = work.tile([128, NZ], F32)
                nc.vector.tensor_scalar(
                    out=t2, in0=z2, scalar1=-2.0, scalar2=1.5,
                    op0=ALU.mult, op1=ALU.add,
                )
                s0 = work.tile([128, NZ], F32)
                nc.vector.tensor_mul(s0, zc, t2)
                s = work.tile([128, NZ], F32)
                nc.vector.tensor_scalar_add(s, s0, 0.5)

                # ---- tree path probabilities ----
                # level l: parents prob_l [128, 2^l, 8], selectors
                # s[:, (2^l-1)*8 : (2^(l+1)-1)*8]; left = p*s, right = p - p*s
                prev = None
                for lvl in range(4):
                    n_par = 1 << lvl
                    cur = work.tile([128, 2 * n_par, K_TREE], F32, tag=f"tree{lvl}")
                    s_l = s[:, (n_par - 1) * K_TREE:(2 * n_par - 1) * K_TREE]
                    s_v = s_l.rearrange("p (n k) -> p n k", k=K_TREE)
                    c_v = cur.rearrange("p (n c) k -> p n c k", c=2)
                    if prev is None:
                        nc.vector.tensor_copy(cur[:, 0, :], s_l)
                        nc.vector.tensor_scalar(
                            out=cur[:, 1, :], in0=s_l, scalar1=-1.0, scalar2=1.0,
                            op0=ALU.mult, op1=ALU.add,
                        )
                    else:
                        nc.vector.tensor_mul(c_v[:, :, 0, :], prev, s_v)
                        nc.vector.tensor_sub(c_v[:, :, 1, :], prev, c_v[:, :, 0, :])
                    prev = cur.rearrange("p (n c) k -> p (n c) k", c=2)
                prob = prev.rearrange("p n k -> p (n k)")  # [128, 128]

                # ---- scores + masked log ----
                lp = work.tile([128, NW], F32)
                nc.scalar.activation(lp, prob, ACTF.Ln, bias=eps_sb, scale=1.0)
                mask = work.tile([128, NW], F32)
                nc.vector.tensor_scalar(
                    out=mask, in0=prob, scalar1=0.0, scalar2=None, op0=ALU.is_gt
                )
                sc2 = work.tile([128, NW], F32)
                nc.vector.tensor_add(sc2, lp, zall[:, NZ:NM])

                # ---- softmax numerator (masked entries zeroed after exp;
                # exact vs reference's -inf since normalization cancels) ----
                rmax = work.tile([128, 1], F32)
                nc.vector.reduce_max(rmax, sc2, axis=mybir.AxisListType.X)
                nmax = work.tile([128, 1], F32)
                nc.vector.tensor_scalar_mul(nmax, rmax, -1.0)
                e0 = work.tile([128, NW], F32)
                nc.scalar.activation(e0, sc2, ACTF.Exp, bias=nmax, scale=1.0)
                e = work.tile([128, NW], F32)
                nc.vector.tensor_mul(e, e0, mask)

                # ---- permutation mix: u[l] = sum_j e[j] pmat[j, l] ----
                u = work.tile([128, N_EXP], F32)
                scr = work.tile([128, NW], F32)
                for l in range(N_EXP):
                    nc.vector.scalar_tensor_tensor(
                        out=scr,
                        in0=e,
                        scalar=1.0,
                        in1=pmat_sb[:, l, :],
                        op0=ALU.bypass,
                        op1=ALU.mult,
                        accum_out=u[:, l:l + 1],
                    )
                usum = work.tile([128, 1], F32)
                nc.vector.reduce_sum(usum, u, axis=mybir.AxisListType.X)
                urec = work.tile([128, 1], F32)
                nc.vector.reciprocal(urec, usum)
                g = work.tile([128, N_EXP], F32)
                nc.vector.tensor_scalar_mul(g, u, urec)

                # ---- weighted sum over experts: y = sum_n f_t[:, n, :] g_n ----
                ytile = ypool.tile([128, D_OUT], F32)
                for h in range(D_OUT // DH):
                    dsl = slice(h * DH, (h + 1) * DH)
                    ftile = fpool.tile([128, N_EXP, DH], F32)
                    nc.sync.dma_start(out=ftile, in_=ft[bsl, :, dsl])
                    acc = ytile[:, dsl]
                    nc.vector.tensor_scalar_mul(acc, ftile[:, 0, :], g[:, 0:1])
                    for n in range(1, N_EXP):
                        nc.vector.scalar_tensor_tensor(
                            out=acc,
                            in0=ftile[:, n, :],
                            scalar=g[:, n:n + 1],
                            in1=acc,
                            op0=ALU.mult,
                            op1=ALU.add,
                        )
                nc.sync.dma_start(out=y[bsl, :], in_=ytile)

    nc.finalize()
    return nc


def _prep_inputs(f, x, permutation_weights, Wz, bz, Ww, bw):
    f = np.asarray(f, np.float32)
    x = np.asarray(x, np.float32)
    pw = np.asarray(permutation_weights, np.float32)
    Wz = np.asarray(Wz, np.float32)
    bz = np.asarray(bz, np.float32)
    Ww = np.asarray(Ww, np.float32)
    bw = np.asarray(bw, np.float32)

    ft = np.ascontiguousarray(f.transpose(0, 2, 1))        # [B, N, D]
    xt = x.T                                               # [D, B] (view)
    wall = np.empty((D_IN, NM), np.float32)
    wall[:, :NZ] = Wz.transpose(1, 0, 2).reshape(D_IN, NZ)
    wall[:, NZ:] = Ww.transpose(1, 0, 2).reshape(D_IN, NW)
    biasv = np.concatenate([bz.reshape(NZ), bw.reshape(NW)]).astype(np.float32)
    # score column j = n*8 + k  ->  pmat[j, l] = P[k, n, l]
    pmat = np.ascontiguousarray(
        pw.transpose(1, 0, 2).reshape(NW, N_EXP)
    )  # [(n,k), l]
    pmatt = np.ascontiguousarray(pmat.T)                   # [l, j] = [16, 128]
    return ft, xt, wall, biasv, pmatt


def kernel(f, x, permutation_weights, Wz, bz, Ww, bw, _trace=False):
    global _CACHED_NC, LAST_RESULTS
    from concourse.bass_utils import run_bass_kernel_spmd

    ft, xt, wall, biasv, pmatt = _prep_inputs(
        f, x, permutation_weights, Wz, bz, Ww, bw
    )

    if _CACHED_NC is None:
        _CACHED_NC = build_nc()
    nc = _CACHED_NC

    in_maps = []
    for c in range(N_CORES):
        rsl = slice(c * BS, (c + 1) * BS)
        xwc = np.concatenate([xt[:, rsl], wall], axis=1)
        in_maps.append(
            {
                "ft": np.ascontiguousarray(ft[rsl]),
                "xw": np.ascontiguousarray(xwc),
                "biasv": biasv,
                "pmatt": pmatt,
            }
        )

    LAST_RESULTS = run_bass_kernel_spmd(
        nc, in_maps, list(range(N_CORES)), trace=_trace
    )
    y = np.concatenate(
        [LAST_RESULTS.results[c]["y"] for c in range(N_CORES)], axis=0
    )
    return y.astype(np.float32)


# revision 15
# speedup vs baseline: 1.0659x; 1.0659x over previous
"""Trainium2 Bass kernel for COMETGate MoE routing.

Computes, for each row b:
    s      = smoothstep(x @ Wz + bz)                       [B, 15, 8]
    prob   = binary-tree path products of s                [B, 16, 8]
    a      = x @ Ww + bw                                   [B, 16, 8]
    score  = a + (prob > 0 ? log(prob + 1e-8) : -inf)      [B, 16*8]
    e      = exp(score - max(score))        (softmax numerator)
    u[l]   = sum_j e[j] * P[j, l]           (permutation mix)
    g      = u / sum(u)                                    [B, 16]
    y[b,d] = sum_n f[b, d, n] * g[b, n]                    [B, 1024]

Sharding: data-parallel over B across 8 NeuronCores (1024 rows each).
Host-side prep: f transposed to [B, N, D] (unit-stride expert slices),
x transposed to [D, B] (matmul stationary wants contraction on
partitions), Wz/Ww fused into one [1024, 248] rhs matrix.
"""

import sys

for _p in ("/opt/trn_rl_repo", "/root/.axon_site/_ro/trn_rl_repo"):
    if _p not in sys.path:
        sys.path.insert(0, _p)

import numpy as np

import concourse.bass as bass
import concourse.tile as tile
from concourse import bacc, mybir

F32 = mybir.dt.float32
ALU = mybir.AluOpType
ACTF = mybir.ActivationFunctionType

B, D_IN, D_OUT = 8192, 1024, 1024
N_EXP, K_TREE = 16, 8
N_CORES = 8
BS = B // N_CORES          # 1024 rows per core
NB = BS // 128             # 8 b-tiles of 128 rows
NZ = (N_EXP - 1) * K_TREE  # 120 selector columns
NW = N_EXP * K_TREE        # 128 leaf columns
NM = NZ + NW               # 248 fused matmul outputs
DH = 512                   # d-chunk for the weighting stage

_CACHED_NC = None
LAST_RESULTS = None  # BassKernelResults of the most recent run (for test.py)


def build_nc():
    nc = bacc.Bacc("TRN2", target_bir_lowering=False, debug=False)

    ft = nc.dram_tensor("ft", [BS, N_EXP, D_OUT], F32, kind="ExternalInput").ap()
    xw = nc.dram_tensor("xw", [D_IN, BS + NM], F32, kind="ExternalInput").ap()
    biasv = nc.dram_tensor("biasv", [NM], F32, kind="ExternalInput").ap()
    pmatt = nc.dram_tensor("pmatt", [N_EXP, NW], F32, kind="ExternalInput").ap()
    y = nc.dram_tensor("y", [BS, D_OUT], F32, kind="ExternalOutput").ap()

    with tile.TileContext(nc) as tc:
        with (
            tc.tile_pool(name="singles", bufs=1) as singles,
            tc.tile_pool(name="work", bufs=2) as work,
            tc.tile_pool(name="fpool", bufs=3) as fpool,
            tc.tile_pool(name="ypool", bufs=2) as ypool,
            tc.tile_pool(name="psum", bufs=2, space="PSUM") as psum,
        ):
            # ---- resident constants ----
            xw_sb = singles.tile([128, D_IN // 128, BS + NM], F32)  # [p, kc, b|m]
            nc.sync.dma_start(
                out=xw_sb, in_=xw.rearrange("(c p) m -> p c m", p=128)
            )
            bias_sb = singles.tile([128, NM], F32)
            bias_ap = biasv[:]
            bias_bc = bass.AP(
                tensor=bias_ap.tensor,
                offset=bias_ap.offset,
                ap=[[0, 128]] + list(bias_ap.ap),
            )
            nc.gpsimd.dma_start(out=bias_sb, in_=bias_bc)
            pmat_sb = singles.tile([128, N_EXP, NW], F32)
            pmat_ap = pmatt[:, :]
            pmat_bc = bass.AP(
                tensor=pmat_ap.tensor,
                offset=pmat_ap.offset,
                ap=[[0, 128]] + list(pmat_ap.ap),
            )
            nc.gpsimd.dma_start(out=pmat_sb, in_=pmat_bc)
            # Wait-absorbers: HW instructions carry a single sync-wait slot,
            # so let DVE observe each input DMA once, up front; later DVE
            # consumers then need no additional cross-engine waits.
            absorb = singles.tile([128, 1], F32)
            nc.vector.tensor_copy(absorb, bias_sb[:, 0:1])
            nc.vector.tensor_copy(absorb, pmat_sb[:, 0, 0:1])
            nc.vector.tensor_copy(absorb, xw_sb[:, 0, 0:1])

            for bt in range(NB):
                bsl = slice(bt * 128, (bt + 1) * 128)

                # ---- routing matmul: scores[b, m] = sum_d x[b,d] W[d,m] ----
                sc_ps = psum.tile([128, NM], F32)
                for kc in range(D_IN // 128):
                    nc.tensor.matmul(
                        sc_ps,
                        xw_sb[:, kc, bsl],
                        xw_sb[:, kc, BS:BS + NM],
                        start=(kc == 0),
                        stop=(kc == D_IN // 128 - 1),
                    )
                zall = work.tile([128, NM], F32)
                nc.vector.tensor_add(zall, sc_ps, bias_sb)

                # ---- smoothstep: s = poly(clamp(z, -.5, .5)) ----
                z = zall[:, 0:NZ]
                zc = work.tile([128, NZ], F32)
                nc.vector.tensor_scalar(
                    out=zc, in0=z, scalar1=-0.5, scalar2=0.5,
                    op0=ALU.max, op1=ALU.min,
                )
                z2 = work.tile([128, NZ], F32)
                nc.vector.tensor_mul(z2, zc, zc)
                t2 = work.tile([128, NZ], F32)
                nc.vector.tensor_scalar(
                    out=t2, in0=z2, scalar1=-2.0, scalar2=1.5,
                    op0=ALU.mult, op1=ALU.add,
                )
                s0 = work.tile([128, NZ], F32)
                nc.vector.tensor_mul(s0, zc, t2)
                s = work.tile([128, NZ], F32)
                nc.vector.tensor_scalar_add(s, s0, 0.5)

                # ---- tree path probabilities ----
                # level l: parents prob_l [128, 2^l, 8], selectors
                # s[:, (2^l-1)*8 : (2^(l+1)-1)*8]; left = p*s, right = p - p*s
                prev = None
                for lvl in range(4):
                    n_par = 1 << lvl
                    cur = work.tile([128, 2 * n_par, K_TREE], F32, tag=f"tree{lvl}")
                    s_l = s[:, (n_par - 1) * K_TREE:(2 * n_par - 1) * K_TREE]
                    s_v = s_l.rearrange("p (n k) -> p n k", k=K_TREE)
                    c_v = cur.rearrange("p (n c) k -> p n c k", c=2)
                    if prev is None:
                        nc.vector.tensor_copy(cur[:, 0, :], s_l)
                        nc.vector.tensor_scalar(
                            out=cur[:, 1, :], in0=s_l, scalar1=-1.0, scalar2=1.0,
                            op0=ALU.mult, op1=ALU.add,
                        )
                    else:
                        nc.vector.tensor_mul(c_v[:, :, 0, :], prev, s_v)
                        nc.vector.tensor_sub(c_v[:, :, 1, :], prev, c_v[:, :, 0, :])
                    prev = cur.rearrange("p (n c) k -> p (n c) k", c=2)
                prob = prev.rearrange("p n k -> p (n k)")  # [128, 128]

                # ---- softmax numerator, log-free form ----
                # exp(a + log(prob+eps) - M) == exp(a - max_a) * (prob+eps)
                # up to a constant factor that the final normalization
                # cancels; masked entries (prob <= 0) are zeroed exactly,
                # matching the reference's -inf scores.
                mask = work.tile([128, NW], F32)
                nc.vector.tensor_scalar(
                    out=mask, in0=prob, scalar1=0.0, scalar2=None, op0=ALU.is_gt
                )
                factor = work.tile([128, NW], F32)
                nc.vector.scalar_tensor_tensor(
                    out=factor, in0=prob, scalar=1e-8, in1=mask,
                    op0=ALU.add, op1=ALU.mult,
                )
                rmax = work.tile([128, 1], F32)
                nc.vector.reduce_max(rmax, zall[:, NZ:NM], axis=mybir.AxisListType.X)
                nmax = work.tile([128, 1], F32)
                nc.vector.tensor_scalar_mul(nmax, rmax, -1.0)
                e0 = work.tile([128, NW], F32)
                nc.scalar.activation(e0, zall[:, NZ:NM], ACTF.Exp, bias=nmax, scale=1.0)
                e = work.tile([128, NW], F32)
                nc.vector.tensor_mul(e, e0, factor)

                # ---- permutation mix: u[l] = sum_j e[j] pmat[j, l] ----
                u = work.tile([128, N_EXP], F32)
                scr = work.tile([128, NW], F32)
                for l in range(N_EXP):
                    nc.vector.scalar_tensor_tensor(
                        out=scr,
                        in0=e,
                        scalar=1.0,
                        in1=pmat_sb[:, l, :],
                        op0=ALU.bypass,
                        op1=ALU.mult,
                        accum_out=u[:, l:l + 1],
                    )
                usum = work.tile([128, 1], F32)
                nc.vector.reduce_sum(usum, u, axis=mybir.AxisListType.X)
                urec = work.tile([128, 1], F32)
                nc.vector.reciprocal(urec, usum)
                g = work.tile([128, N_EXP], F32)
                nc.vector.tensor_scalar_mul(g, u, urec)

                # ---- weighted sum over experts: y = sum_n f_t[:, n, :] g_n ----
                ytile = ypool.tile([128, D_OUT], F32)
                for h in range(D_OUT // DH):
                    dsl = slice(h * DH, (h + 1) * DH)
                    ftile = fpool.tile([128, N_EXP, DH], F32)
                    nc.sync.dma_start(out=ftile, in_=ft[bsl, :, dsl])
                    acc = ytile[:, dsl]
                    nc.vector.tensor_scalar_mul(acc, ftile[:, 0, :], g[:, 0:1])
                    for n in range(1, N_EXP):
                        nc.vector.scalar_tensor_tensor(
                            out=acc,
                            in0=ftile[:, n, :],
                            scalar=g[:, n:n + 1],
                            in1=acc,
                            op0=ALU.mult,
                            op1=ALU.add,
                        )
                nc.sync.dma_start(out=y[bsl, :], in_=ytile)

    nc.finalize()
    return nc


def _prep_inputs(f, x, permutation_weights, Wz, bz, Ww, bw):
    f = np.asarray(f, np.float32)
    x = np.asarray(x, np.float32)
    pw = np.asarray(permutation_weights, np.float32)
    Wz = np.asarray(Wz, np.float32)
    bz = np.asarray(bz, np.float32)
    Ww = np.asarray(Ww, np.float32)
    bw = np.asarray(bw, np.float32)

    ft = np.ascontiguousarray(f.transpose(0, 2, 1))        # [B, N, D]
    xt = x.T                                               # [D, B] (view)
    wall = np.empty((D_IN, NM), np.float32)
    wall[:, :NZ] = Wz.transpose(1, 0, 2).reshape(D_IN, NZ)
    wall[:, NZ:] = Ww.transpose(1, 0, 2).reshape(D_IN, NW)
    biasv = np.concatenate([bz.reshape(NZ), bw.reshape(NW)]).astype(np.float32)
    # score column j = n*8 + k  ->  pmat[j, l] = P[k, n, l]
    pmat = np.ascontiguousarray(
        pw.transpose(1, 0, 2).reshape(NW, N_EXP)
    )  # [(n,k), l]
    pmatt = np.ascontiguousarray(pmat.T)                   # [l, j] = [16, 128]
    return ft, xt, wall, biasv, pmatt


def kernel(f, x, permutation_weights, Wz, bz, Ww, bw, _trace=False):
    global _CACHED_NC, LAST_RESULTS
    from concourse.bass_utils import run_bass_kernel_spmd

    ft, xt, wall, biasv, pmatt = _prep_inputs(
        f, x, permutation_weights, Wz, bz, Ww, bw
    )

    if _CACHED_NC is None:
        _CACHED_NC = build_nc()
    nc = _CACHED_NC

    in_maps = []
    for c in range(N_CORES):
        rsl = slice(c * BS, (c + 1) * BS)
        xwc = np.concatenate([xt[:, rsl], wall], axis=1)
        in_maps.append(
            {
                "ft": np.ascontiguousarray(ft[rsl]),
                "xw": np.ascontiguousarray(xwc),
                "biasv": biasv,
                "pmatt": pmatt,
            }
        )

    LAST_RESULTS = run_bass_kernel_spmd(
        nc, in_maps, list(range(N_CORES)), trace=_trace
    )
    y = np.concatenate(
        [LAST_RESULTS.results[c]["y"] for c in range(N_CORES)], axis=0
    )
    return y.astype(np.float32)
